# revision 7
# baseline (speedup 1.0000x reference)
"""Trainium2 Bass kernel for ConditionalPositionalEncoding1D-style module:
depthwise conv1d(k=3, pad=1) + BatchNorm1d (inference) + multi-step LIF
(tau=2, v_th=1, hard reset) + residual.

Strategy (8 NeuronCores, data-parallel over batch B=32 -> 4 per core):
  * conv+BN folded into 3 accumulating diagonal matmuls on TensorE
    (per-channel weights on the diagonal), bias added by ScalarE
    Identity-activation on the PSUM->SBUF copy. All constants are
    pre-folded on host (including the LIF 1/tau=0.5 pre-scale).
  * LIF scan over T=2048: split into K=16 chunks of L=128 with H=64
    halo steps. v decays by 0.5/step and hard-resets to 0, so a
    chunk started H=64 steps early from v=0 is bit-identical to the
    sequential scan by chunk start (validated empirically: 0 flips).
    All 8 lane-blocks x 16 chunks advance in lockstep -> 192 steps of
    ONE custom fused DVE op each: v' = select(0.5*v + a < 1, ., 0),
    writing v' in place over the consumed `a` value.
  * spikes recovered in bulk: spike == (v' == 0.0) (reset is the only
    way to hit exactly +0.0), fused with the residual via
    scalar_tensor_tensor: out = (v is_eq 0) add x.
"""

import sys

if "/opt/trn_rl_repo" not in sys.path:
    sys.path.insert(0, "/opt/trn_rl_repo")

import numpy as np

import concourse.bass as bass
import concourse.bacc as bacc
import concourse.mybir as mybir
import concourse.tile as tile
import concourse.dve_ops as dve_ops
from concourse.bass_utils import run_bass_kernel_spmd

BN_EPS = 1e-5

# problem geometry (hardcoded per spec)
B, C, T = 32, 256, 2048
NCORES = 8
BP = B // NCORES          # batches per core
P = 128                   # partitions
NLB = BP * (C // P)       # lane blocks per core (b, c-half) = 8
L = 128                   # LIF chunk length
H = 64                    # halo steps
K = T // L                # chunks per lane
S = L + H                 # wavefront steps
TP = T + 2                # x free size (zero col at 0 and T+1)
AT = H + T                # a free size (zero halo cols [0, H))

_lif_op = None


def _get_lif_op():
    """Register the fused LIF-step DVE op (idempotent)."""
    global _lif_op
    if _lif_op is not None:
        return _lif_op
    from concourse.dve_spec import Spec, Src0, Src1, C0, One, Zero, select, lower
    from concourse.dve_uop import DveOpSpec

    u = Src0 * C0 + Src1
    spec = Spec(
        body=select(u < One, u, Zero),
        reference=lambda in0, in1, s0, s1, imm2: (
            lambda u: np.where(u < 1.0, u, 0.0).astype(np.float32)
        )(in0 * s0 + np.asarray(in1).reshape(np.shape(in0))),
    )
    for existing in dve_ops.OPS:
        if existing.name == "LIF_STEP_ANT":
            _lif_op = existing
            return existing
    op = dve_ops.DveOp("LIF_STEP_ANT", spec, subdim=False, uops_sha={})
    dve_ops.OPS.append(op)
    dve_ops._SUB_OPCODE_FOR_NAME[op.name] = (
        dve_ops._CUSTOM_DVE_ROW_BASE + len(dve_ops.OPS) - 1
    )
    dve_ops.CUSTOM_DVE_SPECS[op.name] = op.spec
    for ver in ("v3", "v4"):
        op.uops_sha[ver] = DveOpSpec(
            name=op.name,
            opcode=dve_ops.get_dve_sub_opcode(op.name),
            uops=lower(spec, ver=ver),
            rd1_en=dve_ops.has_src1(spec),
        ).sha(ver)
    _lif_op = op
    return op


def build_program():
    """Build the per-core Bass program (identical on all 8 cores)."""
    lif = _get_lif_op()
    f32 = mybir.dt.float32
    nc = bacc.Bacc(
        "TRN2", target_bir_lowering=False, debug=False, num_devices=NCORES
    )

    x_d = nc.dram_tensor("x", [BP, C, T], f32, kind="ExternalInput")
    wd_d = nc.dram_tensor("wdiag", [P, 6, P], f32, kind="ExternalInput")
    sv_d = nc.dram_tensor("svec", [P, 2], f32, kind="ExternalInput")
    out_d = nc.dram_tensor("out", [BP, C, T], f32, kind="ExternalOutput")

    with tile.TileContext(nc) as tc:
        with (
            tc.tile_pool(name="const", bufs=1) as cpool,
            tc.tile_pool(name="xbuf", bufs=1) as xpool,
            tc.tile_pool(name="abuf", bufs=1) as apool,
            tc.tile_pool(name="state", bufs=1) as spool,
            tc.tile_pool(name="psum", bufs=8, space="PSUM") as ppool,
        ):
            wd_sb = cpool.tile([P, 6, P], f32)
            sv_sb = cpool.tile([P, 2], f32)
            x_sb = xpool.tile([P, NLB, TP], f32)
            a_sb = apool.tile([P, NLB, AT], f32)
            zeros = spool.tile([P, NLB, K], f32)
            scr = [
                spool.tile([P, NLB, K], f32, name=f"scr{i}", tag=f"scr{i}")
                for i in range(2)
            ]

            nc.sync.dma_start(wd_sb[:], wd_d[:])
            nc.sync.dma_start(sv_sb[:], sv_d[:])

            # zero pads
            nc.vector.memset(x_sb[:, :, 0:1], 0.0)
            nc.vector.memset(x_sb[:, :, TP - 1 : TP], 0.0)
            nc.vector.memset(a_sb[:, :, 0:H], 0.0)
            nc.vector.memset(zeros[:], 0.0)

            # ---- Phase A: load x, conv+BN via diagonal matmuls ----
            for lb in range(NLB):
                b, h = divmod(lb, C // P)
                nc.sync.dma_start(
                    x_sb[:, lb, 1 : T + 1], x_d[b, h * P : (h + 1) * P, :]
                )
            NTT = T // 512
            for lb in range(NLB):
                b, h = divmod(lb, C // P)
                for tt in range(NTT):
                    ps = ppool.tile([P, 512], f32)
                    for k in range(3):
                        nc.tensor.matmul(
                            ps[:],
                            wd_sb[:, k * 2 + h, :],
                            x_sb[:, lb, tt * 512 + k : tt * 512 + k + 512],
                            start=(k == 0),
                            stop=(k == 2),
                        )
                    nc.scalar.activation(
                        a_sb[:, lb, H + tt * 512 : H + (tt + 1) * 512],
                        ps[:],
                        mybir.ActivationFunctionType.Identity,
                        bias=sv_sb[:, h : h + 1],
                        scale=1.0,
                    )

            # ---- Phase B: LIF wavefront, 192 fused steps ----
            for s in range(S):
                in0 = zeros[:] if s == 0 else (
                    scr[(s - 1) % 2][:] if s <= H else
                    a_sb[:, :, s - 1 : s - 1 + (K - 1) * L + 1 : L]
                )
                out_ap = (
                    scr[s % 2][:] if s < H else a_sb[:, :, s : s + (K - 1) * L + 1 : L]
                )
                nc.vector._custom_dve(
                    lif,
                    out=out_ap,
                    in0=in0,
                    in1=a_sb[:, :, s : s + (K - 1) * L + 1 : L],
                    s0=0.5,
                )

            # ---- Phase C: spikes + residual, in place over x ----
            for lb in range(NLB):
                nc.vector.scalar_tensor_tensor(
                    x_sb[:, lb, 1 : T + 1],
                    a_sb[:, lb, H : H + T],
                    0.0,
                    x_sb[:, lb, 1 : T + 1],
                    mybir.AluOpType.is_equal,
                    mybir.AluOpType.add,
                )

            # ---- store ----
            for lb in range(NLB):
                b, h = divmod(lb, C // P)
                nc.sync.dma_start(
                    out_d[b, h * P : (h + 1) * P, :], x_sb[:, lb, 1 : T + 1]
                )
    nc.finalize()
    return nc


def _host_constants(conv_w, conv_b, gamma, beta, run_mean, run_var):
    f32 = np.float32
    inv = (np.asarray(gamma, f32)
           / np.sqrt(np.asarray(run_var, f32) + f32(BN_EPS))).astype(f32)
    wt = (np.asarray(conv_w, f32)[:, 0, :] * inv[:, None] * f32(0.5)).astype(f32)
    st = ((np.asarray(conv_b, f32) * inv + np.asarray(beta, f32)
           - np.asarray(run_mean, f32) * inv) * f32(0.5)).astype(f32)
    wdiag = np.zeros((P, 6, P), f32)
    svec = np.zeros((P, 2), f32)
    rng = np.arange(P)
    for tap in range(3):
        for h in range(2):
            wdiag[rng, tap * 2 + h, rng] = wt[h * P : (h + 1) * P, tap]
    for h in range(2):
        svec[:, h] = st[h * P : (h + 1) * P]
    return wdiag, svec


def run(inputs, trace=False):
    x = np.ascontiguousarray(np.asarray(inputs["x"], np.float32))
    wdiag, svec = _host_constants(
        inputs["conv_w"], inputs["conv_b"], inputs["gamma"],
        inputs["beta"], inputs["run_mean"], inputs["run_var"],
    )
    nc = build_program()
    in_maps = [
        {
            "x": np.ascontiguousarray(x[i * BP : (i + 1) * BP]),
            "wdiag": wdiag,
            "svec": svec,
        }
        for i in range(NCORES)
    ]
    res = run_bass_kernel_spmd(nc, in_maps, list(range(NCORES)), trace=trace)
    out = np.concatenate([res.results[i]["out"] for i in range(NCORES)], axis=0)
    return out, res


def kernel(**inputs):
    out, _ = run(inputs)
    return out


# revision 15
# speedup vs baseline: 1.3546x; 1.3546x over previous
"""Trainium2 Bass kernel for ConditionalPositionalEncoding1D-style module:
depthwise conv1d(k=3, pad=1) + BatchNorm1d (inference) + multi-step LIF
(tau=2, v_th=1, hard reset) + residual.

Strategy (8 NeuronCores, data-parallel over batch B=32 -> 4 per core):
  * conv+BN folded into 3 accumulating diagonal matmuls on TensorE
    (per-channel weights on the diagonal), bias added by ScalarE
    Identity-activation on the PSUM->SBUF copy. All constants are
    pre-folded on host (including the LIF 1/tau=0.5 pre-scale).
  * LIF scan over T=2048: split into K=16 chunks of L=128 with H=64
    halo steps. v decays by 0.5/step and hard-resets to 0, so a
    chunk started H=64 steps early from v=0 is bit-identical to the
    sequential scan by chunk start (validated empirically: 0 flips).
    All 8 lane-blocks x 16 chunks advance in lockstep -> 192 steps of
    ONE custom fused DVE op each: v' = select(0.5*v + a < 1, ., 0),
    writing v' in place over the consumed `a` value.
  * spikes recovered in bulk: spike == (v' == 0.0) (reset is the only
    way to hit exactly +0.0), fused with the residual via
    scalar_tensor_tensor: out = (v is_eq 0) add x.
"""

import sys

if "/opt/trn_rl_repo" not in sys.path:
    sys.path.insert(0, "/opt/trn_rl_repo")

import numpy as np

import concourse.bass as bass
import concourse.bacc as bacc
import concourse.mybir as mybir
import concourse.tile as tile
import concourse.dve_ops as dve_ops
from concourse.bass_utils import run_bass_kernel_spmd

BN_EPS = 1e-5

# problem geometry (hardcoded per spec)
B, C, T = 32, 256, 2048
NCORES = 8
BP = B // NCORES          # batches per core
P = 128                   # partitions
NLB = BP * (C // P)       # lane blocks per core (b, c-half) = 8
L = 128                   # LIF chunk length
H = 48                    # halo steps (validated: 0 flips vs H=64/sequential)
NPE = 4                   # lane-blocks convolved on TensorE; the rest on DVE
K = T // L                # chunks per lane
S = L + H                 # wavefront steps
TP = T + 2                # x free size (zero col at 0 and T+1)
AT = H + T                # a free size (zero halo cols [0, H))

_lif_op = None


def _get_lif_op():
    """Register the fused LIF-step DVE op (idempotent)."""
    global _lif_op
    if _lif_op is not None:
        return _lif_op
    from concourse.dve_spec import Spec, Src0, Src1, C0, One, Zero, select, lower
    from concourse.dve_uop import DveOpSpec

    u = Src0 * C0 + Src1
    spec = Spec(
        body=select(u < One, u, Zero),
        reference=lambda in0, in1, s0, s1, imm2: (
            lambda u: np.where(u < 1.0, u, 0.0).astype(np.float32)
        )(in0 * s0 + np.asarray(in1).reshape(np.shape(in0))),
    )
    for existing in dve_ops.OPS:
        if existing.name == "LIF_STEP_ANT":
            _lif_op = existing
            return existing
    op = dve_ops.DveOp("LIF_STEP_ANT", spec, subdim=False, uops_sha={})
    dve_ops.OPS.append(op)
    dve_ops._SUB_OPCODE_FOR_NAME[op.name] = (
        dve_ops._CUSTOM_DVE_ROW_BASE + len(dve_ops.OPS) - 1
    )
    dve_ops.CUSTOM_DVE_SPECS[op.name] = op.spec
    for ver in ("v3", "v4"):
        op.uops_sha[ver] = DveOpSpec(
            name=op.name,
            opcode=dve_ops.get_dve_sub_opcode(op.name),
            uops=lower(spec, ver=ver),
            rd1_en=dve_ops.has_src1(spec),
        ).sha(ver)
    _lif_op = op
    return op


def build_program():
    """Build the per-core Bass program (identical on all 8 cores)."""
    lif = _get_lif_op()
    f32 = mybir.dt.float32
    nc = bacc.Bacc(
        "TRN2", target_bir_lowering=False, debug=False, num_devices=NCORES
    )

    x_d = nc.dram_tensor("x", [BP, C, T], f32, kind="ExternalInput")
    wd_d = nc.dram_tensor("wdiag", [P, 6, P], f32, kind="ExternalInput")
    wv_d = nc.dram_tensor("wvec", [P, 6], f32, kind="ExternalInput")
    sv_d = nc.dram_tensor("svec", [P, 2], f32, kind="ExternalInput")
    out_d = nc.dram_tensor("out", [BP, C, T], f32, kind="ExternalOutput")

    with tile.TileContext(nc) as tc:
        with (
            tc.tile_pool(name="const", bufs=1) as cpool,
            tc.tile_pool(name="xbuf", bufs=1) as xpool,
            tc.tile_pool(name="abuf", bufs=1) as apool,
            tc.tile_pool(name="state", bufs=1) as spool,
            tc.tile_pool(name="psum", bufs=8, space="PSUM") as ppool,
        ):
            wd_sb = cpool.tile([P, 6, P], f32)
            wv_sb = cpool.tile([P, 6], f32)
            sv_sb = cpool.tile([P, 2], f32)
            x_sb = xpool.tile([P, NLB, TP], f32)
            a_sb = apool.tile([P, NLB, AT], f32)
            zeros = spool.tile([P, NLB, K], f32)
            scr = [
                spool.tile([P, NLB, K], f32, name=f"scr{i}", tag=f"scr{i}")
                for i in range(2)
            ]

            nc.sync.dma_start(wd_sb[:], wd_d[:])
            nc.sync.dma_start(wv_sb[:], wv_d[:])
            nc.sync.dma_start(sv_sb[:], sv_d[:])

            # zero pads
            nc.vector.memset(x_sb[:, :, 0:1], 0.0)
            nc.vector.memset(x_sb[:, :, TP - 1 : TP], 0.0)
            nc.vector.memset(a_sb[:, :, 0:H], 0.0)
            nc.vector.memset(zeros[:], 0.0)

            # ---- Phase A: load x, conv+BN via diagonal matmuls ----
            for lb in range(NLB):
                b, h = divmod(lb, C // P)
                nc.sync.dma_start(
                    x_sb[:, lb, 1 : T + 1], x_d[b, h * P : (h + 1) * P, :]
                )
            NTT = T // 512
            for lb in range(NPE):
                b, h = divmod(lb, C // P)
                for tt in range(NTT):
                    ps = ppool.tile([P, 512], f32)
                    for k in range(3):
                        nc.tensor.matmul(
                            ps[:],
                            wd_sb[:, k * 2 + h, :],
                            x_sb[:, lb, tt * 512 + k : tt * 512 + k + 512],
                            start=(k == 0),
                            stop=(k == 2),
                        )
                    nc.scalar.activation(
                        a_sb[:, lb, H + tt * 512 : H + (tt + 1) * 512],
                        ps[:],
                        mybir.ActivationFunctionType.Identity,
                        bias=sv_sb[:, h : h + 1],
                        scale=1.0,
                    )
            # remaining lane-blocks on DVE, same accumulation order as the PE
            # path (w0*xm1 + w1*x + w2*xp1, bias last) so results match bitwise
            for lb in range(NPE, NLB):
                b, h = divmod(lb, C // P)
                dst = a_sb[:, lb, H : H + T]
                nc.vector.tensor_scalar(
                    dst, x_sb[:, lb, 0:T],
                    wv_sb[:, h : h + 1], None, mybir.AluOpType.mult,
                )
                nc.vector.scalar_tensor_tensor(
                    dst, x_sb[:, lb, 1 : T + 1], wv_sb[:, 2 + h : 3 + h], dst,
                    mybir.AluOpType.mult, mybir.AluOpType.add,
                )
                nc.vector.scalar_tensor_tensor(
                    dst, x_sb[:, lb, 2 : T + 2], wv_sb[:, 4 + h : 5 + h], dst,
                    mybir.AluOpType.mult, mybir.AluOpType.add,
                )
                nc.vector.tensor_scalar(
                    dst, dst, sv_sb[:, h : h + 1], None, mybir.AluOpType.add,
                )

            # ---- Phase B: LIF wavefront, 192 fused steps ----
            for s in range(S):
                in0 = zeros[:] if s == 0 else (
                    scr[(s - 1) % 2][:] if s <= H else
                    a_sb[:, :, s - 1 : s - 1 + (K - 1) * L + 1 : L]
                )
                out_ap = (
                    scr[s % 2][:] if s < H else a_sb[:, :, s : s + (K - 1) * L + 1 : L]
                )
                nc.vector._custom_dve(
                    lif,
                    out=out_ap,
                    in0=in0,
                    in1=a_sb[:, :, s : s + (K - 1) * L + 1 : L],
                    s0=0.5,
                )

            # ---- Phase C: spikes + residual, in place over x ----
            for lb in range(NLB):
                nc.vector.scalar_tensor_tensor(
                    x_sb[:, lb, 1 : T + 1],
                    a_sb[:, lb, H : H + T],
                    0.0,
                    x_sb[:, lb, 1 : T + 1],
                    mybir.AluOpType.is_equal,
                    mybir.AluOpType.add,
                )

            # ---- store ----
            for lb in range(NLB):
                b, h = divmod(lb, C // P)
                nc.sync.dma_start(
                    out_d[b, h * P : (h + 1) * P, :], x_sb[:, lb, 1 : T + 1]
                )
    nc.finalize()
    return nc


def _host_constants(conv_w, conv_b, gamma, beta, run_mean, run_var):
    f32 = np.float32
    inv = (np.asarray(gamma, f32)
           / np.sqrt(np.asarray(run_var, f32) + f32(BN_EPS))).astype(f32)
    wt = (np.asarray(conv_w, f32)[:, 0, :] * inv[:, None] * f32(0.5)).astype(f32)
    st = ((np.asarray(conv_b, f32) * inv + np.asarray(beta, f32)
           - np.asarray(run_mean, f32) * inv) * f32(0.5)).astype(f32)
    wdiag = np.zeros((P, 6, P), f32)
    wvec = np.zeros((P, 6), f32)
    svec = np.zeros((P, 2), f32)
    rng = np.arange(P)
    for tap in range(3):
        for h in range(2):
            wdiag[rng, tap * 2 + h, rng] = wt[h * P : (h + 1) * P, tap]
            wvec[:, tap * 2 + h] = wt[h * P : (h + 1) * P, tap]
    for h in range(2):
        svec[:, h] = st[h * P : (h + 1) * P]
    return wdiag, wvec, svec


def run(inputs, trace=False):
    x = np.ascontiguousarray(np.asarray(inputs["x"], np.float32))
    wdiag, wvec, svec = _host_constants(
        inputs["conv_w"], inputs["conv_b"], inputs["gamma"],
        inputs["beta"], inputs["run_mean"], inputs["run_var"],
    )
    nc = build_program()
    in_maps = [
        {
            "x": np.ascontiguousarray(x[i * BP : (i + 1) * BP]),
            "wdiag": wdiag,
            "wvec": wvec,
            "svec": svec,
        }
        for i in range(NCORES)
    ]
    res = run_bass_kernel_spmd(nc, in_maps, list(range(NCORES)), trace=trace)
    out = np.concatenate([res.results[i]["out"] for i in range(NCORES)], axis=0)
    return out, res


def kernel(**inputs):
    out, _ = run(inputs)
    return out
